# revision 1
# baseline (speedup 1.0000x reference)
"""Trainium2 Bass kernel for DiffCompressModule.

Reference computation (B=4, S=512, D_IN=D_OUT=4096):
    out = h @ W.T + b + coeff[b] * (h @ (2*mask[b] - 1))

Fused form used here (one matmul instead of two):
    out[b] = h[b] @ M_b + bias,   M_b = W.T + coeff[b] * (2*mask[b] - 1)

M_b is built in bf16 on ACT+DVE while the 256MB int32 mask streams from
HBM; the matmul runs in bf16 with fp32 PSUM accumulation. The kernel is
HBM-bound (~68MB per core).

Sharding over 8 cores: 4 out-feature groups x 2 batch groups.
Each core: h [2,512,4096], W [1024,4096], bias [1024], coeff [2],
mask [2,4096,1024] -> out [2,512,1024].
"""

import numpy as np

import concourse.bass as bass
import concourse.mybir as mybir
from concourse import tile, masks
from concourse.bass_utils import run_bass_kernel_spmd

B, S, D = 4, 512, 4096
O_FULL = 4096
N_CORES = 8
OG, BG = 4, 2  # out-feature groups x batch groups
O_SH = O_FULL // OG  # 1024 out features per core
B_SH = B // BG  # 2 batches per core
HALF = 512  # o processed in halves (PSUM/SBUF budget)
KC = D // 128  # 32 contraction chunks
SC = S // 128  # 4 s chunks
dt = mybir.dt

_CACHE = {}


def _split_sync_waits(nc, max_waits=1):
    # CoreV3 walrus rejects instructions with more than one semaphore wait
    # ("Too many sync wait commands"). Splitting the waits across preceding
    # same-engine NOPs is equivalent (the sequencer blocks on each in turn).
    ctr = 0
    for fn in nc.m.functions:
        for bb in fn.blocks:
            insts = bb.instructions
            if not any(
                i.sync_info is not None and len(i.sync_info.on_wait) > max_waits
                for i in insts
            ):
                continue
            new_list = []
            for ins in insts:
                si = ins.sync_info
                if si is not None and len(si.on_wait) > max_waits:
                    waits = list(si.on_wait)
                    head, tail = waits[:-max_waits], waits[-max_waits:]
                    for k in range(0, len(head), max_waits):
                        nop = mybir.InstNoOp(
                            name=f"waitsplit-{ctr}",
                            engine=ins.engine,
                            ins=[],
                            outs=[],
                            sync_info=mybir.SyncInfo(
                                on_wait=head[k : k + max_waits], on_update=[]
                            ),
                        )
                        ctr += 1
                        new_list.append(nop)
                    ins.sync_info = mybir.SyncInfo(
                        on_wait=tail, on_update=list(si.on_update)
                    )
                new_list.append(ins)
            bb.instructions = new_list


def _build_nc(loop_n=None):
    nc = bass.Bass("TRN2", target_bir_lowering=False, debug=False)
    h = nc.dram_tensor("h", [B_SH, D, S], dt.float32, kind="ExternalInput").ap()
    W = nc.dram_tensor("W", [D, O_SH], dt.float32, kind="ExternalInput").ap()
    bias = nc.dram_tensor("bias", [O_SH], dt.float32, kind="ExternalInput").ap()
    coeff = nc.dram_tensor("coeff", [B_SH], dt.float32, kind="ExternalInput").ap()
    mask = nc.dram_tensor("mask", [B_SH, D, O_SH], dt.int32, kind="ExternalInput").ap()
    out = nc.dram_tensor("out", [B_SH, S, O_SH], dt.float32, kind="ExternalOutput").ap()

    with tile.TileContext(nc) as tc:
        with (
            tc.tile_pool(name="const", bufs=1) as const_pool,
            tc.tile_pool(name="wt", bufs=KC // 2 + 1) as wt_pool,
            tc.tile_pool(name="ht", bufs=B_SH * KC // 4) as ht_pool,
            tc.tile_pool(name="mk", bufs=5) as mk_pool,
            tc.tile_pool(name="tt", bufs=2) as t_pool,
            tc.tile_pool(name="m", bufs=KC + 2) as m_pool,
            tc.tile_pool(name="ost", bufs=4) as out_pool,
            tc.tile_pool(name="acc", bufs=8, space="PSUM") as acc_pool,
        ):
            bias_bc = const_pool.tile([128, O_SH], dt.float32)
            nc.sync.dma_start(
                bias_bc[:], bass.AP(bias.tensor, 0, [[0, 128], [1, O_SH]])
            )
            coeff_bc = const_pool.tile([128, B_SH], dt.float32)
            nc.sync.dma_start(
                coeff_bc[:], bass.AP(coeff.tensor, 0, [[0, 128], [1, B_SH]])
            )
            c2 = const_pool.tile([128, B_SH], dt.float32)
            cneg = const_pool.tile([128, B_SH], dt.float32)
            nc.vector.tensor_scalar_mul(c2[:], coeff_bc[:], 2.0)
            nc.vector.tensor_scalar_mul(cneg[:], coeff_bc[:], -1.0)

            ht = {}

            import contextlib

            loop_ctx = (
                tc.For_i(
                    0,
                    loop_n,
                    1,
                    hint_engines=(
                        mybir.EngineType.PE,
                        mybir.EngineType.Activation,
                        mybir.EngineType.DVE,
                        mybir.EngineType.SP,
                        mybir.EngineType.Pool,
                    ),
                )
                if loop_n
                else contextlib.nullcontext()
            )

            def build_ht_kg(b, kg):
                # h arrives pre-transposed [b, i, s] from the host: one
                # casting SWDGE DMA fills 4 kc chunks ([128 i, 4*512 s] bf16)
                if kg % 2 == 1:
                    return
                for q in range(4):  # 4 quads cover kc in [kg*8, kg*8+16)
                    kc0 = kg * 8 + q * 4
                    ht4 = ht_pool.tile([128, 4 * S], dt.bfloat16, name="ht4")
                    nc.gpsimd.dma_start(
                        ht4[:],
                        bass.AP(
                            h.tensor,
                            (b * D + kc0 * 128) * S,
                            [[S, 128], [128 * S, 4], [1, S]],
                        ),
                    )
                    for j in range(4):
                        ht[(b, kc0 + j)] = ht4[:, j * S : (j + 1) * S]

            def build_wt_kg(half, kg, wt):
                # W arrives pre-transposed [i, o] from the host: one casting
                # SWDGE DMA fills 4 kc chunks ([128 i, 4*512 o] bf16)
                o0 = half * HALF
                for q in range(2):  # 2 quads cover kc in [kg*8, kg*8+8)
                    kc0 = kg * 8 + q * 4
                    wt4 = wt_pool.tile([128, 4 * HALF], dt.bfloat16, name="wt4")
                    nc.gpsimd.dma_start(
                        wt4[:],
                        bass.AP(
                            W.tensor,
                            kc0 * 128 * O_SH + o0,
                            [[O_SH, 128], [128 * O_SH, 4], [1, HALF]],
                        ),
                    )
                    for j in range(4):
                        wt.append(wt4[:, j * HALF : (j + 1) * HALF])

            def round_kg(half, b, kg, wt, accs):
                o0 = half * HALF
                for k2 in range(2):  # quads of kc chunks, cast int32->bf16 in DMA
                    kc0 = kg * 8 + k2 * 4
                    mk = mk_pool.tile([128, 4 * HALF], dt.bfloat16, name="mk")
                    nc.gpsimd.dma_start(
                        mk[:],
                        bass.AP(
                            mask.tensor,
                            (b * D + kc0 * 128) * O_SH + o0,
                            [[O_SH, 128], [128 * O_SH, 4], [1, HALF]],
                        ),
                    )
                    t_sb = t_pool.tile([128, 4 * HALF], dt.bfloat16, name="tsb")
                    nc.scalar.activation(
                        t_sb[:],
                        mk[:],
                        mybir.ActivationFunctionType.Identity,
                        bias=cneg[:, b : b + 1],
                        scale=c2[:, b : b + 1],
                    )
                    for j in range(4):
                        kc = kc0 + j
                        m = m_pool.tile([128, HALF], dt.bfloat16, name="m")
                        nc.vector.tensor_tensor(
                            m[:],
                            t_sb[:, j * HALF : (j + 1) * HALF],
                            wt[kc][:],
                            mybir.AluOpType.add,
                        )
                        for sc in range(SC):
                            htap = ht[(b, kc)]
                            nc.tensor.matmul(
                                accs[sc][:],
                                htap[:, sc * 128 : (sc + 1) * 128],
                                m[:],
                                start=(kc == 0),
                                stop=(kc == KC - 1),
                            )

            def epilogue(half, b, accs):
                o0 = half * HALF
                for sc in range(SC):
                    o_sb = out_pool.tile([128, HALF], dt.float32, name="osb")
                    nc.vector.tensor_tensor(
                        o_sb[:],
                        accs[sc][:],
                        bias_bc[:, o0 : o0 + HALF],
                        mybir.AluOpType.add,
                    )
                    nc.sync.dma_start(
                        out[b, sc * 128 : (sc + 1) * 128, o0 : o0 + HALF], o_sb[:]
                    )

            def new_accs():
                return [
                    acc_pool.tile([128, HALF], dt.float32, tag="acc", name="acc")
                    for _ in range(SC)
                ]

            with loop_ctx:
                wt0, wt1 = [], []
                accs = new_accs()
                for kg in range(4):
                    build_ht_kg(0, kg)
                    build_wt_kg(0, kg, wt0)
                    round_kg(0, 0, kg, wt0, accs)
                    build_wt_kg(1, kg, wt1)
                epilogue(0, 0, accs)
                accs = new_accs()
                for kg in range(4):
                    round_kg(1, 0, kg, wt1, accs)
                    build_ht_kg(1, kg)
                epilogue(1, 0, accs)
                accs = new_accs()
                for kg in range(4):
                    round_kg(0, 1, kg, wt0, accs)
                epilogue(0, 1, accs)
                accs = new_accs()
                for kg in range(4):
                    round_kg(1, 1, kg, wt1, accs)
                epilogue(1, 1, accs)

    _split_sync_waits(nc)
    return nc


def _get_nc():
    if "nc" not in _CACHE:
        _CACHE["nc"] = _build_nc()
    return _CACHE["nc"]


def kernel(hidden_states, W, b, coeff, mask, _trace=False, _trace_kwargs=None):
    nc = _get_nc()
    in_maps = []
    for core in range(N_CORES):
        g, bj = core // BG, core % BG
        in_maps.append(
            {
                "h": np.ascontiguousarray(
                    np.asarray(hidden_states)[
                        bj * B_SH : (bj + 1) * B_SH
                    ].transpose(0, 2, 1),
                    dtype=np.float32,
                ),
                "W": np.ascontiguousarray(
                    np.asarray(W)[g * O_SH : (g + 1) * O_SH].T, dtype=np.float32
                ),
                "bias": np.ascontiguousarray(
                    b[g * O_SH : (g + 1) * O_SH], dtype=np.float32
                ),
                "coeff": np.ascontiguousarray(
                    coeff[bj * B_SH : (bj + 1) * B_SH], dtype=np.float32
                ),
                "mask": np.ascontiguousarray(
                    mask[bj * B_SH : (bj + 1) * B_SH, :, g * O_SH : (g + 1) * O_SH],
                    dtype=np.int32,
                ),
            }
        )
    kwargs = {}
    if _trace:
        kwargs = {"trace": True, "trace_kwargs": _trace_kwargs or {}}
    # The first touch of the device after an abnormal process exit can fail
    # with NRT_EXEC_UNIT_UNRECOVERABLE; the failed attempt clears the wedged
    # state, so retry.
    last_err = None
    for attempt in range(3):
        try:
            res = run_bass_kernel_spmd(
                nc, in_maps, core_ids=list(range(N_CORES)), **kwargs
            )
            break
        except Exception as e:  # jax.errors.JaxRuntimeError etc.
            last_err = e
            try:
                import jax

                jax.clear_caches()
            except Exception:
                pass
            import time as _time

            _time.sleep(2.0)
    else:
        raise last_err
    _CACHE["last_results"] = res

    out = np.empty((B, S, O_FULL), dtype=np.float32)
    for core in range(N_CORES):
        g, bj = core // BG, core % BG
        out[bj * B_SH : (bj + 1) * B_SH, :, g * O_SH : (g + 1) * O_SH] = res.results[
            core
        ]["out"]
    return out

